# revision 2
# baseline (speedup 1.0000x reference)
"""Trainium2 Bass kernel for nn_Attention_68015102099893 (sparse_attention).

B=2048 independent 9x9 attention blocks over patch tokens, fc 512->256.
Data parallel over 8 cores (256 batches each); 14 batches per group so
(batch, patch) = 126 rows sit on SBUF partitions (padded to 128 with the
next group's rows); per-batch 9x9 attention is block-diagonal math on
128x128 tiles driven by a host-precomputed additive mask.

v5 (on top of v3's math shortcuts):
 - attn2 = softmax(attn@attn/3) is EXACTLY the identity (the -100 diag
   mask gives the second-order logits a ~850 gap), so the second softmax
   input is gram*scale - 99*eye; E = exp(logits) is symmetric and feeds
   the values matmul as the stationary (E @ v), no PE transpose needed.
 - v3 read x TWICE (row-major for values + c-major for the fc), making
   the kernel HBM-bound at ~3.52 MB/group (measured 363 GB/s aggregate
   over 16 DMA engines -> 202 us). v5 reads the row-major copy plus only
   24/36 of the c-major tiles from HBM; the other 12 tiles are produced
   on-chip by PE transposes (is_transpose matmul with an identity moving
   operand). Each transpose costs ~97ns of LDWEIGHTS cadence on the PE's
   single FWL port (measured; depth-1 shadow buffer means every matmul
   pays max(stream, 97ns)), so the split is chosen to balance
   PE ~8.7us/group against DMA ~8.6us/group.
 - FC column chunks are 3x384 (not 512/512/128) so every FC matmul
   stream (160ns) exceeds the 97ns ldweights shadow.
"""

import os
import numpy as np

PS = 3
N = 9           # patches per image
P = 9           # tokens per patch
B = 2048
C = 512
HID = 256
NCORES = 8
BLOC = B // NCORES            # 256 batches per core
ROWS = BLOC * N               # 2304 (b, n) rows per core
G = 14                        # batches per group
GR = G * N                    # 126 rows per group
FULL_GROUPS = BLOC // G       # 18
REM = BLOC - FULL_GROUPS * G  # 4 remaining batches
ROWS_PAD = ROWS + 128         # DRAM pad so every group can load 128 rows
NG = FULL_GROUPS + (1 if REM else 0)  # 19 groups per core
CPG = P * C                   # 4608 columns per row
SCALE = float((HID * P) ** -0.5)
NEG = -1.0e30

NT = 12                       # (j,p) tiles transposed on the PE per group
NH = 36 - NT                  # tiles read c-major from HBM
TCOL = NT * 128               # 1536 xts columns produced by transposes
HCOL = NH * 128               # 3072 xts columns DMA'd from HBM

_CACHE = {}


def _groups():
    gs = [(g * GR, GR) for g in range(FULL_GROUPS)]
    if REM:
        gs.append((FULL_GROUPS * GR, REM * N))
    return gs


def _build():
    import concourse.bacc as bacc
    import concourse.tile as tile
    from concourse import mybir

    BF = mybir.dt.bfloat16
    F32 = mybir.dt.float32
    Copy = mybir.ActivationFunctionType.Copy
    Ident = mybir.ActivationFunctionType.Identity
    Exp = mybir.ActivationFunctionType.Exp
    MUL = mybir.AluOpType.mult
    ADD = mybir.AluOpType.add

    nc = bacc.Bacc("TRN2", target_bir_lowering=False)

    xv = nc.dram_tensor("xv", [ROWS_PAD, CPG], BF, kind="ExternalInput")
    xt = nc.dram_tensor("xt", [NG * 128, HCOL], BF, kind="ExternalInput")
    w4 = nc.dram_tensor("w4", [128, 4 * HID], BF, kind="ExternalInput")
    b2 = nc.dram_tensor("b2", [128, 2], F32, kind="ExternalInput")
    mf = nc.dram_tensor("mf", [128, 128], F32, kind="ExternalInput")
    mr = nc.dram_tensor("mr", [128, 128], F32, kind="ExternalInput")
    idn = nc.dram_tensor("idn", [128, 128], BF, kind="ExternalInput")
    out = nc.dram_tensor("out", [ROWS, CPG], BF, kind="ExternalOutput")

    groups = _groups()
    ng = len(groups)
    # FC column chunks over the 1152 (p, m) columns of one c-chunk
    FCCH = [(0, 384), (384, 384), (768, 384)]

    with tile.TileContext(nc) as tc:
        with (
            tc.tile_pool(name="const", bufs=1) as cpool,
            tc.tile_pool(name="vt", bufs=6) as vpool,
            tc.tile_pool(name="xt", bufs=4) as tpool,
            tc.tile_pool(name="xq", bufs=2) as qpool,
            tc.tile_pool(name="small", bufs=3) as spool,
            tc.tile_pool(name="outs", bufs=3) as opool,
            tc.tile_pool(name="psfc", bufs=2, space="PSUM") as psfc_pool,
            tc.tile_pool(name="pstp", bufs=2, space="PSUM") as pstp_pool,
            tc.tile_pool(name="psg", bufs=2, space="PSUM") as psg_pool,
            tc.tile_pool(name="pso", bufs=2, space="PSUM") as pso_pool,
        ):
            st = {}

            def emit_load(g):
                r0, _ = groups[g]
                # 128 partition rows (2 pad rows of real next-group data):
                # 126-row DMAs measured much slower (descriptor swizzle).
                vT = vpool.tile([128, CPG], BF, tag="vT")
                nc.sync.dma_start(out=vT[:, :], in_=xv[r0:r0 + 128, :])
                st[g] = {"vT": vT}

            def emit_ct(g):
                # c-major tiles 12..35 straight from HBM on the scalar ring
                s = st[g]
                xts = tpool.tile([128, CPG], BF, tag="xts")
                nc.scalar.dma_start(
                    out=xts[:, TCOL:], in_=xt[g * 128:(g + 1) * 128, :]
                )
                s["xts"] = xts

            def emit_trans(g):
                # c-major tiles 0..11 via PE transpose (4 per psum bank)
                s = st[g]
                vT, xts = s["vT"], s["xts"]
                for q in range(NT // 4):
                    pst = pstp_pool.tile([128, 512], BF, tag="pstp")
                    for k in range(4):
                        t = q * 4 + k
                        nc.tensor.transpose(
                            pst[:, k * 128:(k + 1) * 128],
                            vT[:, t * 128:(t + 1) * 128],
                            idnS[:, :],
                        )
                    nc.vector.tensor_copy(
                        out=xts[:, q * 512:(q + 1) * 512], in_=pst[:, :]
                    )

            def emit_fc(g):
                s = st[g]
                xts = s["xts"]
                xq = qpool.tile([128, 2 * 1152], BF, tag="xq")
                for h in range(2):
                    for ci, (ts0, tw) in enumerate(FCCH):
                        ps = psfc_pool.tile([128, 384], F32, tag="psfc")
                        for j in range(4):
                            nc.tensor.matmul(
                                ps[:, :tw],
                                lhsT=wS[:, j * HID + h * 128:j * HID + h * 128 + 128],
                                rhs=xts[:, j * 1152 + ts0:j * 1152 + ts0 + tw],
                                start=(j == 0),
                                stop=(j == 3),
                            )
                        dst = xq[:, h * 1152 + ts0:h * 1152 + ts0 + tw]
                        if (h * 3 + ci) % 2 == 0:
                            nc.vector.tensor_scalar_add(
                                out=dst, in0=ps[:, :tw], scalar1=bS[:, h:h + 1],
                            )
                        else:
                            nc.scalar.activation(
                                dst, ps[:, :tw], Ident,
                                bias=bS[:, h:h + 1], scale=1.0,
                            )
                s["xq"] = xq

            def emit_attn(g):
                s = st[g]
                xq = s["xq"]
                psgt = psg_pool.tile([128, 128], F32, tag="psg")
                for k in range(18):
                    h, p = divmod(k, 9)
                    sl = xq[:, h * 1152 + p * 128:h * 1152 + (p + 1) * 128]
                    nc.tensor.matmul(
                        psgt[:, :128], lhsT=sl, rhs=sl,
                        start=(k == 0), stop=(k == 17),
                    )
                lg = spool.tile([128, 128], F32, tag="lg")
                mS = mrS if g == ng - 1 and REM else mfS
                nc.vector.scalar_tensor_tensor(
                    out=lg[:, :], in0=psgt[:, :128], scalar=SCALE,
                    in1=mS[:, :], op0=MUL, op1=ADD,
                )
                E = spool.tile([128, 128], BF, tag="E")
                sm = spool.tile([128, 1], F32, tag="sm")
                nc.scalar.activation(
                    E[:, :], lg[:, :], Exp, scale=1.0, accum_out=sm[:, :],
                )
                ri = spool.tile([128, 1], F32, tag="ri")
                nc.vector.reciprocal(ri[:, :], sm[:, :])
                s["E"] = E
                s["ri"] = ri

            def emit_av(g):
                s = st[g]
                r0, rows = groups[g]
                E, ri, vT = s["E"], s["ri"], s["vT"]
                outsb = opool.tile([128, CPG], BF, tag="outsb")
                for dd in range(9):
                    pso = pso_pool.tile([128, 512], F32, tag="pso")
                    nc.tensor.matmul(
                        pso[:, :],
                        lhsT=E[:, :],
                        rhs=vT[:, dd * 512:(dd + 1) * 512],
                        start=True, stop=True,
                    )
                    dst = outsb[:rows, dd * 512:(dd + 1) * 512]
                    if dd % 2 == 0:
                        nc.scalar.activation(
                            dst, pso[:rows, :], Copy, scale=ri[:rows],
                        )
                    else:
                        nc.vector.tensor_scalar(
                            out=dst, in0=pso[:rows, :],
                            scalar1=ri[:rows], scalar2=None, op0=MUL,
                        )
                # final (REM) write on the fast HWDGE ring: SWDGE's ~1us
                # first-byte latency would sit at the very end of the kernel
                if g == ng - 1:
                    weng = nc.sync
                else:
                    weng = (nc.gpsimd, nc.sync)[g % 2]
                weng.dma_start(out=out[r0:r0 + rows, :], in_=outsb[:rows, :])
                del st[g]

            # first loads lead both rings; consts follow (needed only by
            # the first transpose/FC at ~10us, well after they land)
            emit_load(0)
            emit_ct(0)
            emit_load(1)
            emit_ct(1)
            wS = cpool.tile([128, 4 * HID], BF)
            nc.sync.dma_start(out=wS[:, :], in_=w4[:, :])
            bS = cpool.tile([128, 2], F32)
            nc.sync.dma_start(out=bS[:, :], in_=b2[:, :])
            mfS = cpool.tile([128, 128], F32)
            mrS = cpool.tile([128, 128], F32)
            idnS = cpool.tile([128, 128], BF)
            nc.sync.dma_start(out=mfS[:, :], in_=mf[:, :])
            nc.sync.dma_start(out=mrS[:, :], in_=mr[:, :])
            nc.sync.dma_start(out=idnS[:, :], in_=idn[:, :])
            emit_load(2)
            emit_ct(2)
            emit_load(3)
            for g in range(ng):
                if g + 4 < ng:
                    emit_load(g + 4)
                if g + 3 < ng:
                    emit_ct(g + 3)
                emit_trans(g)
                emit_fc(g)
                emit_attn(g)
                emit_av(g)

    nc.finalize()
    return nc


def _host_prep(x, W_fc, b_fc):
    from concourse import mybir

    bf16 = mybir.dt.np(mybir.dt.bfloat16)
    # patch view: token order (b, n=(n1,n2)), patch-local (p=(p1,p2))
    xfc = x.reshape(B, PS, PS, PS, PS, C).transpose(0, 1, 3, 2, 4, 5)
    xfc = np.ascontiguousarray(xfc).reshape(B, N * P, C)

    # column order (j, p, cc): c split into 4 chunks of 128
    w4 = np.ascontiguousarray(
        W_fc.T.reshape(4, 128, HID).transpose(1, 0, 2).reshape(128, 4 * HID)
    ).astype(bf16)
    b2 = np.ascontiguousarray(b_fc.reshape(2, 128).T).astype(np.float32)

    def mask(nblk):
        m = np.full((128, 128), NEG, np.float32)
        r = nblk * N
        blk = np.kron(np.eye(nblk, dtype=np.float32), np.ones((N, N), np.float32))
        m[:r, :r] = np.where(blk > 0, 0.0, NEG)
        idx = np.arange(r)
        m[idx, idx] = -99.0
        return m

    mfA = mask(G)
    mrA = mask(REM)
    idn = np.eye(128, dtype=bf16)

    in_maps = []
    for i in range(NCORES):
        sh = xfc[i * BLOC:(i + 1) * BLOC].reshape(ROWS, P, 4, 128)
        xv_i = np.zeros((ROWS_PAD, CPG), dtype=bf16)
        xv_i[:ROWS] = sh.transpose(0, 2, 1, 3).reshape(ROWS, CPG).astype(bf16)
        # c-major per-group blocks for (j,p) tiles NT..35 only:
        #   xt[g*128+cc, (t-NT)*128 + m] = x[g*126+m, p, j*128+cc], t=j*9+p
        xt_i = np.empty((NG, 128, HCOL), dtype=bf16)
        xv4 = xv_i.reshape(ROWS_PAD, 36, 128)  # [m, t=(j,p), cc]
        for g in range(NG):
            r0 = g * GR
            xt_i[g] = (
                xv4[r0:r0 + 128, NT:, :].transpose(2, 1, 0).reshape(128, HCOL)
            )
        in_maps.append({
            "xv": xv_i, "xt": xt_i.reshape(NG * 128, HCOL),
            "w4": w4, "b2": b2, "mf": mfA, "mr": mrA, "idn": idn,
        })
    return in_maps


def kernel(x, W_fc, b_fc):
    from concourse.bass_utils import run_bass_kernel_spmd

    x = np.asarray(x, dtype=np.float32)
    W_fc = np.asarray(W_fc, dtype=np.float32)
    b_fc = np.asarray(b_fc, dtype=np.float32)

    if "nc" not in _CACHE:
        _CACHE["nc"] = _build()
    nc = _CACHE["nc"]
    in_maps = _host_prep(x, W_fc, b_fc)

    trace = bool(int(os.environ.get("KERNEL_TRACE", "0")))
    res = run_bass_kernel_spmd(
        nc, in_maps, core_ids=list(range(NCORES)), trace=trace
    )
    _CACHE["last_result"] = res

    outs = []
    for r in res.results:
        o = np.asarray(r["out"], dtype=np.float32)          # [ROWS, (j,p,cc)]
        o = o.reshape(ROWS, 4, P, 128).transpose(0, 2, 1, 3)  # [ROWS, p, c]
        outs.append(o.reshape(BLOC, N, P, C))
    o = np.concatenate(outs, axis=0)                         # [B, N, P, C]
    o = o.reshape(B, PS, PS, PS, PS, C).transpose(0, 1, 3, 2, 4, 5)
    return np.ascontiguousarray(o.reshape(B, N, N, C))
